# revision 21
# baseline (speedup 1.0000x reference)
"""Trainium2 Bass kernel for nn_ExpectedKLDivergence.

Data-parallel over batch across 8 cores. The pairwise expected-KL term is
algebraically reduced (verified vs f64) to

    total = first + T1 - C2*T2
    T1 = sum_s P[s-1]*A[s],   T2 = sum_s Q[s-1]*Q[s]
    A  = p0*(ln p0 - C1) + p1*(ln p1 - C1),  P = p0+p1,  Q = p0-p1
    C1 = (ln b + ln(1-b))/2,  C2 = (ln b - ln(1-b))/2

The ragged mask is a per-row prefix, so the host packs only the valid
prefixes of each row into dense streams per core (rows balanced across
cores by total valid length), with eps separators making cross-row pair
terms vanish. Per rep the device does: ONE big DMA of [x0 || x1] bf16 (SP
HWDGE ring) + one fp8 DMA of the host-computed Q=x0-x1 plane (ACT HWDGE
ring, so the two transfers run on separate FIFO rings); ONE full-width Ln
activation; TWO DVE ops (af = x*lc multiplied in place into lc, then
a = af0+af1); and the tensor engine does all shifted multiply+reduce work
via a diagonal-accumulation trick: for 128-col chunks,
psum[128,128] += past_chunk.T @ curr_chunk accumulates sum_s past[s]*curr[s]
on its diagonal (extracted once per launch, traced on the host in f64).
This keeps every DVE operand 4B-aligned (the +1 shift lives in the PE
operand reads, measured ~4ns/MM extra). Host combines partials in f64 and
divides by B.
"""

import numpy as np
import ml_dtypes

import concourse.bacc as bacc
import concourse.mybir as mybir
import concourse.tile as tile
from concourse.bass_utils import run_bass_kernel_spmd

ALPHA = 0.1
BETA = 0.9
B, S = 512, 32768
NCORES = 8
P = 128                      # partitions
MMCH = 128                   # PE diagonal-trick chunk width
EPS = 1e-6                   # row separator / padding value

C1 = float((np.log(BETA) + np.log(1.0 - BETA)) / 2.0)
C2 = float((np.log(BETA) - np.log(1.0 - BETA)) / 2.0)
ESC = float(np.exp(-C1))     # Ln(x*ESC) = ln(x) - C1

BF = ml_dtypes.bfloat16
F8 = ml_dtypes.float8_e4m3
_BUILT: dict = {}            # width -> compiled Bacc module


def _build(width: int, reps: int = 1, unroll: bool = False, variant: str = "full"):
    f32 = mybir.dt.float32
    bf = mybir.dt.bfloat16
    f8 = mybir.dt.float8e4
    Ln = mybir.ActivationFunctionType.Ln
    add = mybir.AluOpType.add
    W1 = width + 2               # one plane width (halo + stream + pad)
    WS = 2 * W1                  # [x0 || x1]

    nc = bacc.Bacc()
    xd = nc.dram_tensor("xcat", [P, WS], bf, kind="ExternalInput")
    qd = nc.dram_tensor("qtc", [P, W1], f8, kind="ExternalInput")
    f0d = nc.dram_tensor("f0", [P, 2], bf, kind="ExternalInput")
    outd = nc.dram_tensor("acc", [P, 2 * MMCH + 4], f32, kind="ExternalOutput")

    with tile.TileContext(nc) as tc:
        with (
            tc.tile_pool(name="io", bufs=2) as io,
            tc.tile_pool(name="lcp", bufs=2) as lcp,
            tc.tile_pool(name="wk", bufs=2) as wk,
            tc.tile_pool(name="cs", bufs=1) as cs,
            tc.tile_pool(name="psp", bufs=1, space="PSUM") as psp,
        ):
            ps1 = psp.tile([P, MMCH], f32, tag="ps1")
            ps2 = psp.tile([P, MMCH], f32, tag="ps2")
            NCH = width // MMCH

            # warm the ACT Ln table before the loop so walrus doesn't place
            # the ~2.7us PSEUDO_LOAD_ACT_FUNC_SET inside the loop body
            warm = cs.tile([P, 2], bf, tag="warm")
            nc.gpsimd.memset(warm[:], 0.5)
            nc.scalar.activation(warm[:], warm[:], Ln, scale=ESC)

            from contextlib import nullcontext
            loop_ctx = (
                tc.For_i(0, reps, 1) if reps > 1 and not unroll else nullcontext()
            )
            with loop_ctx:
              for _rep in range(reps if unroll else 1):
                x = io.tile([P, WS], bf, tag="x")
                nc.sync.dma_start(x[:], xd[:])
                qt = io.tile([P, W1], f8, tag="qt")
                nc.scalar.dma_start(qt[:], qd[:])

                lc = lcp.tile([P, WS], bf, tag="lc")
                nc.scalar.activation(lc[:], x[:], Ln, scale=ESC)
                # af = x * lc, multiplied in place into lc (saves a buffer)
                nc.vector.tensor_mul(lc[:], x[:], lc[:])
                a = wk.tile([P, W1], bf, tag="a")
                nc.vector.tensor_add(a[:], lc[:, 0:W1], lc[:, W1:WS])

                for ci in range(NCH):
                    c = ci * MMCH
                    first = ci == 0
                    last = ci == NCH - 1
                    nc.tensor.matmul(
                        ps1[:], x[:, c : c + MMCH], a[:, c + 1 : c + 1 + MMCH],
                        start=first, stop=False,
                    )
                    nc.tensor.matmul(
                        ps1[:], x[:, W1 + c : W1 + c + MMCH],
                        a[:, c + 1 : c + 1 + MMCH],
                        start=False, stop=last,
                    )
                    nc.tensor.matmul(
                        ps2[:], qt[:, c : c + MMCH], qt[:, c + 1 : c + 1 + MMCH],
                        start=first, stop=last,
                    )

            # epilogue (once per launch): PSUM copy-out + alpha terms; the
            # host takes the diagonal traces in f64
            outsb = cs.tile([P, 2 * MMCH + 4], f32, tag="outsb")
            nc.gpsimd.memset(outsb[:], 0.0)
            nc.vector.tensor_copy(outsb[:, 0:MMCH], ps1[:])
            nc.vector.tensor_copy(outsb[:, MMCH : 2 * MMCH], ps2[:])

            t0 = cs.tile([P, 2], bf, tag="t0")
            nc.sync.dma_start(t0[:], f0d[:])
            l0 = cs.tile([P, 2], bf, tag="l0")
            nc.scalar.activation(l0[:, 0:1], t0[:, 0:1], Ln, scale=1.0 / (1.0 - ALPHA))
            nc.scalar.activation(l0[:, 1:2], t0[:, 1:2], Ln, scale=1.0 / ALPHA)
            e3 = cs.tile([P, 2], f32, tag="e3")
            nc.vector.tensor_mul(e3[:], t0[:], l0[:])
            nc.vector.tensor_reduce(
                outsb[:, 2 * MMCH : 2 * MMCH + 1], e3[:], mybir.AxisListType.X, add
            )
            nc.sync.dma_start(outd[:], outsb[:])
    nc.compile()
    return nc


def _assign_rows(lengths):
    """Greedy LPT balance of rows across cores by packed size (len+1)."""
    order = np.argsort(-lengths)
    loads = np.zeros(NCORES, np.int64)
    rows = [[] for _ in range(NCORES)]
    for r in order:
        c = int(np.argmin(loads))
        rows[c].append(int(r))
        loads[c] += int(lengths[r]) + 1
    return rows, loads


def _pack_plane(plane_rows, width):
    """Pack a list of (row f32 arrays) into [P, width+2] with halo/pad."""
    flat = np.full(P * width, EPS, np.float32)
    pos = 0
    for seg in plane_rows:
        L = seg.shape[0]
        flat[pos : pos + L] = seg
        pos += L + 1                          # eps separator
    arr = np.empty((P, width + 2), np.float32)
    arr[:, 1 : width + 1] = flat.reshape(P, width)
    arr[0, 0] = EPS                           # virtual past for first row
    arr[1:, 0] = arr[:-1, width]              # halo: previous flat element
    arr[:, width + 1] = EPS                   # lookahead pad (never a current)
    return arr


def _prep_core(p0, p1, lengths, rows, width):
    """Pack valid prefixes of `rows` into bf16 [x0||x1] + fp8 Q streams."""
    arr0 = _pack_plane([p0[r, : lengths[r]] for r in rows], width)
    arr1 = _pack_plane([p1[r, : lengths[r]] for r in rows], width)
    xcat = np.empty((P, 2 * (width + 2)), BF)
    xcat[:, 0 : width + 2] = arr0.astype(BF)
    xcat[:, width + 2 :] = arr1.astype(BF)
    qtc = (arr0 - arr1).astype(F8)
    f0 = np.empty((P, 2), np.float32)
    f0[:, 0] = 1.0 - ALPHA                    # pad rows contribute exactly 0
    f0[:, 1] = ALPHA
    nr = len(rows)
    f0[:nr, 0] = p0[rows, 0]
    f0[:nr, 1] = p1[rows, 0]
    return {"xcat": xcat, "qtc": qtc, "f0": f0.astype(BF)}


def kernel(posterior, length):
    post = np.asarray(posterior, dtype=np.float32)
    ln = np.asarray(length).astype(np.int64)
    assert post.shape == (B, S, 2), post.shape
    lengths = np.clip(ln, 1, S)

    p0 = np.ascontiguousarray(post[..., 0])
    p1 = np.ascontiguousarray(post[..., 1])
    rows, loads = _assign_rows(lengths)
    # common packed width per partition, rounded up to MMCH granularity
    wmax = int(np.ceil(loads.max() / P))
    width = max(MMCH, -(-wmax // MMCH) * MMCH)

    in_maps = [
        _prep_core(p0, p1, lengths, rows[c], width) for c in range(NCORES)
    ]

    if width not in _BUILT:
        _BUILT[width] = _build(width)
    res = run_bass_kernel_spmd(_BUILT[width], in_maps, core_ids=list(range(NCORES)))

    total = np.float64(0.0)
    for c, r in enumerate(res.results):
        acc = np.asarray(r["acc"], np.float64)
        t1 = np.trace(acc[:, 0:MMCH])
        t2 = np.trace(acc[:, MMCH : 2 * MMCH])
        total += t1 - C2 * t2 + acc[: len(rows[c]), 2 * MMCH].sum()
    return np.float32(total / B)


# revision 22
# speedup vs baseline: 1.6804x; 1.6804x over previous
"""Trainium2 Bass kernel for nn_ExpectedKLDivergence.

Data-parallel over batch across 8 cores. The pairwise expected-KL term is
algebraically reduced (verified vs f64) to

    total = first + T1 - C2*T2
    T1 = sum_s P[s-1]*A[s],   T2 = sum_s Q[s-1]*Q[s]
    A  = p0*(ln p0 - C1) + p1*(ln p1 - C1),  P = p0+p1,  Q = p0-p1
    C1 = (ln b + ln(1-b))/2,  C2 = (ln b - ln(1-b))/2

The ragged mask is a per-row prefix, so the host packs only the valid
prefixes of each row into dense streams per core (rows balanced across
cores by total valid length), with eps separators making cross-row pair
terms vanish. Work is tiled in TN-column slabs, deeply multi-buffered so
several reps stay in flight; slab DMAs alternate between the SP and ACT
HWDGE rings (per-engine FIFO). Per slab: one Ln activation over [x0||x1],
af = x*lc multiplied in place into lc, a = af0+af1; Q comes precomputed
from the host as fp8. The tensor engine does all shifted multiply+reduce
work via a diagonal-accumulation trick: psum[128,128] += past.T @ curr per
128-col chunk accumulates sum_s past[s]*curr[s] on its diagonal (extracted
once per launch, traced on the host in f64). Every DVE operand stays
4B-aligned (the +1 shift lives in the PE operand reads, ~4ns/MM).
"""

import numpy as np
import ml_dtypes

import concourse.bacc as bacc
import concourse.mybir as mybir
import concourse.tile as tile
from concourse.bass_utils import run_bass_kernel_spmd

ALPHA = 0.1
BETA = 0.9
B, S = 512, 32768
NCORES = 8
P = 128                      # partitions
TN = 2048                    # stream columns per tile
MMCH = 128                   # PE diagonal-trick chunk width
EPS = 1e-6                   # row separator / padding value

C1 = float((np.log(BETA) + np.log(1.0 - BETA)) / 2.0)
C2 = float((np.log(BETA) - np.log(1.0 - BETA)) / 2.0)
ESC = float(np.exp(-C1))     # Ln(x*ESC) = ln(x) - C1

BF = ml_dtypes.bfloat16
F8 = ml_dtypes.float8_e4m3
_BUILT: dict = {}            # width -> compiled Bacc module


def _tile_sizes(width: int):
    assert width % MMCH == 0
    sizes = [TN] * (width // TN)
    if width % TN:
        sizes.append(width % TN)
    return sizes


def _build(width: int, reps: int = 1, unroll: bool = False, variant: str = "full"):
    f32 = mybir.dt.float32
    bf = mybir.dt.bfloat16
    f8 = mybir.dt.float8e4
    Ln = mybir.ActivationFunctionType.Ln
    add = mybir.AluOpType.add
    sizes = _tile_sizes(width)
    NT = len(sizes)
    cofs = [sum(2 * (s + 2) for s in sizes[:i]) for i in range(NT)]
    qofs = [sum(s + 2 for s in sizes[:i]) for i in range(NT)]
    total_cols = sum(2 * (s + 2) for s in sizes)
    qt_cols = sum(s + 2 for s in sizes)

    nc = bacc.Bacc()
    xd = nc.dram_tensor("xcat", [P, total_cols], bf, kind="ExternalInput")
    qd = nc.dram_tensor("qtc", [P, qt_cols], f8, kind="ExternalInput")
    f0d = nc.dram_tensor("f0", [P, 2], bf, kind="ExternalInput")
    outd = nc.dram_tensor("acc", [P, 2 * MMCH + 4], f32, kind="ExternalOutput")

    with tile.TileContext(nc) as tc:
        with (
            tc.tile_pool(name="io", bufs=8) as io,
            tc.tile_pool(name="qp", bufs=6) as qp,
            tc.tile_pool(name="lcp", bufs=6) as lcp,
            tc.tile_pool(name="wk", bufs=6) as wk,
            tc.tile_pool(name="cs", bufs=1) as cs,
            tc.tile_pool(name="psp", bufs=1, space="PSUM") as psp,
        ):
            ps1 = psp.tile([P, MMCH], f32, tag="ps1")
            ps2 = psp.tile([P, MMCH], f32, tag="ps2")

            # warm the ACT Ln table before the loop so walrus doesn't place
            # the ~2.7us PSEUDO_LOAD_ACT_FUNC_SET inside the loop body
            warm = cs.tile([P, 2], bf, tag="warm")
            nc.gpsimd.memset(warm[:], 0.5)
            nc.scalar.activation(warm[:], warm[:], Ln, scale=ESC)

            from contextlib import nullcontext
            loop_ctx = (
                tc.For_i(0, reps, 1) if reps > 1 and not unroll else nullcontext()
            )
            with loop_ctx:
              for _rep in range(reps if unroll else 1):
               for k in range(NT):
                NK = sizes[k]
                W1 = NK + 2                     # one plane's slab width
                WS = 2 * W1                     # [x0 || x1] slab width
                dma_eng = nc.sync if k % 2 == 0 else nc.scalar
                x = io.tile([P, WS], bf, tag="x")
                dma_eng.dma_start(x[:], xd[:, cofs[k] : cofs[k] + WS])
                qt = qp.tile([P, W1], f8, tag="qt")
                (nc.scalar if k % 2 == 0 else nc.sync).dma_start(
                    qt[:], qd[:, qofs[k] : qofs[k] + W1]
                )

                lc = lcp.tile([P, WS], bf, tag="lc")
                nc.scalar.activation(lc[:], x[:], Ln, scale=ESC)
                # af = x * lc, multiplied in place into lc (saves a buffer)
                nc.vector.tensor_mul(lc[:], x[:], lc[:])
                a = wk.tile([P, W1], bf, tag="a")
                nc.vector.tensor_add(a[:], lc[:, 0:W1], lc[:, W1:WS])

                last_c = NK - MMCH
                for c in range(0, NK, MMCH):
                    first = k == 0 and c == 0
                    last = k == NT - 1 and c == last_c
                    nc.tensor.matmul(
                        ps1[:], x[:, c : c + MMCH], a[:, c + 1 : c + 1 + MMCH],
                        start=first, stop=False,
                    )
                    nc.tensor.matmul(
                        ps1[:], x[:, W1 + c : W1 + c + MMCH],
                        a[:, c + 1 : c + 1 + MMCH],
                        start=False, stop=last,
                    )
                    nc.tensor.matmul(
                        ps2[:], qt[:, c : c + MMCH], qt[:, c + 1 : c + 1 + MMCH],
                        start=first, stop=last,
                    )

            # epilogue (once per launch): PSUM copy-out + alpha terms; the
            # host takes the diagonal traces in f64
            outsb = cs.tile([P, 2 * MMCH + 4], f32, tag="outsb")
            nc.gpsimd.memset(outsb[:], 0.0)
            nc.vector.tensor_copy(outsb[:, 0:MMCH], ps1[:])
            nc.vector.tensor_copy(outsb[:, MMCH : 2 * MMCH], ps2[:])

            t0 = cs.tile([P, 2], bf, tag="t0")
            nc.sync.dma_start(t0[:], f0d[:])
            l0 = cs.tile([P, 2], bf, tag="l0")
            nc.scalar.activation(l0[:, 0:1], t0[:, 0:1], Ln, scale=1.0 / (1.0 - ALPHA))
            nc.scalar.activation(l0[:, 1:2], t0[:, 1:2], Ln, scale=1.0 / ALPHA)
            e3 = cs.tile([P, 2], f32, tag="e3")
            nc.vector.tensor_mul(e3[:], t0[:], l0[:])
            nc.vector.tensor_reduce(
                outsb[:, 2 * MMCH : 2 * MMCH + 1], e3[:], mybir.AxisListType.X, add
            )
            nc.sync.dma_start(outd[:], outsb[:])
    nc.compile()
    return nc


def _assign_rows(lengths):
    """Greedy LPT balance of rows across cores by packed size (len+1)."""
    order = np.argsort(-lengths)
    loads = np.zeros(NCORES, np.int64)
    rows = [[] for _ in range(NCORES)]
    for r in order:
        c = int(np.argmin(loads))
        rows[c].append(int(r))
        loads[c] += int(lengths[r]) + 1
    return rows, loads


def _pack_plane(plane_rows, width):
    """Pack a list of row f32 arrays into [P, width+2] with halo/pad."""
    flat = np.full(P * width, EPS, np.float32)
    pos = 0
    for seg in plane_rows:
        L = seg.shape[0]
        flat[pos : pos + L] = seg
        pos += L + 1                          # eps separator
    arr = np.empty((P, width + 2), np.float32)
    arr[:, 1 : width + 1] = flat.reshape(P, width)
    arr[0, 0] = EPS                           # virtual past for first row
    arr[1:, 0] = arr[:-1, width]              # halo: previous flat element
    arr[:, width + 1] = EPS                   # lookahead pad (never a current)
    return arr


def _prep_core(p0, p1, lengths, rows, width):
    """Pack valid prefixes of `rows` into per-tile bf16 [x0||x1] slabs plus
    a host-computed fp8 Q=x0-x1 stream (tiles overlap by the 2 halo cols)."""
    arr0 = _pack_plane([p0[r, : lengths[r]] for r in rows], width)
    arr1 = _pack_plane([p1[r, : lengths[r]] for r in rows], width)
    arrq = arr0 - arr1
    sizes = _tile_sizes(width)
    total_cols = sum(2 * (s + 2) for s in sizes)
    qt_cols = sum(s + 2 for s in sizes)
    xcat = np.empty((P, total_cols), BF)
    qtc = np.empty((P, qt_cols), F8)
    st = co = qo = 0
    for NK in sizes:
        xcat[:, co : co + NK + 2] = arr0[:, st : st + NK + 2].astype(BF)
        xcat[:, co + NK + 2 : co + 2 * NK + 4] = arr1[:, st : st + NK + 2].astype(BF)
        qtc[:, qo : qo + NK + 2] = arrq[:, st : st + NK + 2].astype(F8)
        st += NK
        co += 2 * NK + 4
        qo += NK + 2
    f0 = np.empty((P, 2), np.float32)
    f0[:, 0] = 1.0 - ALPHA                    # pad rows contribute exactly 0
    f0[:, 1] = ALPHA
    nr = len(rows)
    f0[:nr, 0] = p0[rows, 0]
    f0[:nr, 1] = p1[rows, 0]
    return {"xcat": xcat, "qtc": qtc, "f0": f0.astype(BF)}


def kernel(posterior, length):
    post = np.asarray(posterior, dtype=np.float32)
    ln = np.asarray(length).astype(np.int64)
    assert post.shape == (B, S, 2), post.shape
    lengths = np.clip(ln, 1, S)

    p0 = np.ascontiguousarray(post[..., 0])
    p1 = np.ascontiguousarray(post[..., 1])
    rows, loads = _assign_rows(lengths)
    # common packed width per partition, rounded up to MMCH granularity
    wmax = int(np.ceil(loads.max() / P))
    width = max(MMCH, -(-wmax // MMCH) * MMCH)

    in_maps = [
        _prep_core(p0, p1, lengths, rows[c], width) for c in range(NCORES)
    ]

    if width not in _BUILT:
        _BUILT[width] = _build(width)
    res = run_bass_kernel_spmd(_BUILT[width], in_maps, core_ids=list(range(NCORES)))

    total = np.float64(0.0)
    for c, r in enumerate(res.results):
        acc = np.asarray(r["acc"], np.float64)
        t1 = np.trace(acc[:, 0:MMCH])
        t2 = np.trace(acc[:, MMCH : 2 * MMCH])
        total += t1 - C2 * t2 + acc[: len(rows[c]), 2 * MMCH].sum()
    return np.float32(total / B)


# revision 27
# speedup vs baseline: 1.8586x; 1.1060x over previous
"""Trainium2 Bass kernel for nn_ExpectedKLDivergence.

Data-parallel over batch across 8 cores. The pairwise expected-KL term is
algebraically reduced (verified vs f64) to

    total = first + T1 - C2*T2
    T1 = sum_s P[s-1]*A[s],   T2 = sum_s Q[s-1]*Q[s]
    A  = p0*(ln p0 - C1) + p1*(ln p1 - C1),  P = p0+p1,  Q = p0-p1
    C1 = (ln b + ln(1-b))/2,  C2 = (ln b - ln(1-b))/2

The ragged mask is a per-row prefix, so the host packs only the valid
prefixes of each row into dense streams per core (rows balanced across
cores by total valid length), with eps separators making cross-row pair
terms vanish.

Per rep the device issues exactly TWO DMAs — one per HWDGE ring (SP and
ACT) — each landing a byte-typed group buffer that holds [x0|x1] bf16
slabs for its tiles followed by the host-precomputed Q=x0-x1 fp8 slabs
(per-DMA fixed cost ~2us dominates with many small transfers, so transfers
are merged; dtype mixing is handled by slice+bitcast views). Per group:
one full-width Ln activation and one in-place af=x*lc multiply; per tile:
one a=af0+af1 add. The tensor engine does all shifted multiply+reduce work
via a diagonal-accumulation trick: psum[128,128] += past.T @ curr per
128-col chunk accumulates sum_s past[s]*curr[s] on its diagonal (extracted
once per launch, traced on the host in f64). Every DVE operand stays
4B-aligned (the +1 shift lives in the PE operand reads, ~4ns/MM). The Ln
ACT table is warmed before the loop so the ~2.7us table load stays out of
the body, and several reps are unrolled inside the hardware loop to cut
For_i boundary costs.
"""

import numpy as np
import ml_dtypes

import concourse.bacc as bacc
import concourse.mybir as mybir
import concourse.tile as tile
from concourse.bass_utils import run_bass_kernel_spmd

ALPHA = 0.1
BETA = 0.9
B, S = 512, 32768
NCORES = 8
P = 128                      # partitions
TN = 2048                    # stream columns per tile
MMCH = 128                   # PE diagonal-trick chunk width
EPS = 1e-6                   # row separator / padding value

C1 = float((np.log(BETA) + np.log(1.0 - BETA)) / 2.0)
C2 = float((np.log(BETA) - np.log(1.0 - BETA)) / 2.0)
ESC = float(np.exp(-C1))     # Ln(x*ESC) = ln(x) - C1

BF = ml_dtypes.bfloat16
F8 = ml_dtypes.float8_e4m3
_BUILT: dict = {}            # width -> compiled Bacc module


def _layout(width: int):
    """Tile sizes and byte offsets of the two DMA group buffers.

    Group g holds, per partition: [x0_t|x1_t bf16 (4*W1t bytes) for t in g]
    then [qt_t fp8 (W1q_t bytes, padded to 4) for t in g].
    Returns (sizes, groups, xoff, qoff, gbytes) with offsets group-relative.
    """
    assert width % MMCH == 0
    sizes = [TN] * (width // TN)
    if width % TN:
        sizes.append(width % TN)
    NT = len(sizes)
    half = (NT + 1) // 2
    groups = [list(range(half)), list(range(half, NT))]
    if not groups[1]:
        groups[1] = []
    xoff, qoff = {}, {}
    gbytes = []
    for g in groups:
        off = 0
        for t in g:
            xoff[t] = off
            off += 4 * (sizes[t] + 2)
        for t in g:
            qoff[t] = off
            off += -(-(sizes[t] + 2) // 4) * 4      # fp8 slab padded to 4B
        gbytes.append(off)
    return sizes, groups, xoff, qoff, gbytes


def _build(
    width: int,
    reps: int = 1,
    unroll: bool = False,
    variant: str = "full",
    inner: int = 1,
):
    f32 = mybir.dt.float32
    bf = mybir.dt.bfloat16
    f8 = mybir.dt.float8e4
    Ln = mybir.ActivationFunctionType.Ln
    add = mybir.AluOpType.add
    sizes, groups, xoff, qoff, gbytes = _layout(width)

    nc = bacc.Bacc()
    gAd = nc.dram_tensor("gA", [P, gbytes[0]], f8, kind="ExternalInput")
    gBd = (
        nc.dram_tensor("gB", [P, gbytes[1]], f8, kind="ExternalInput")
        if gbytes[1]
        else None
    )
    f0d = nc.dram_tensor("f0", [P, 2], bf, kind="ExternalInput")
    outd = nc.dram_tensor("acc", [P, 2 * MMCH + 4], f32, kind="ExternalOutput")

    with tile.TileContext(nc) as tc:
        with (
            tc.tile_pool(name="io", bufs=2) as io,
            tc.tile_pool(name="lcp", bufs=3) as lcp,
            tc.tile_pool(name="wk", bufs=4) as wk,
            tc.tile_pool(name="cs", bufs=1) as cs,
            tc.tile_pool(name="psp", bufs=1, space="PSUM") as psp,
        ):
            ps1 = psp.tile([P, MMCH], f32, tag="ps1")
            ps2 = psp.tile([P, MMCH], f32, tag="ps2")

            # warm the ACT Ln table before the loop so walrus doesn't place
            # the ~2.7us PSEUDO_LOAD_ACT_FUNC_SET inside the loop body
            warm = cs.tile([P, 2], bf, tag="warm")
            nc.gpsimd.memset(warm[:], 0.5)
            nc.scalar.activation(warm[:], warm[:], Ln, scale=ESC)

            from contextlib import nullcontext
            loop_ctx = (
                tc.For_i(0, reps, 1) if reps > 1 and not unroll else nullcontext()
            )
            with loop_ctx:
              for _rep in range(reps if unroll else inner):
                bufs = []
                for gi, g in enumerate(groups):
                    if not g:
                        bufs.append(None)
                        continue
                    gb = io.tile([P, gbytes[gi]], f8, tag=f"g{gi}", name=f"g{gi}")
                    eng = nc.sync if gi == 0 else nc.scalar
                    eng.dma_start(gb[:], (gAd if gi == 0 else gBd)[:])
                    bufs.append(gb)

                lcg = []
                for gi, g in enumerate(groups):
                    if not g:
                        lcg.append(None)
                        continue
                    xb = sum(4 * (sizes[t] + 2) for t in g)      # x-region bytes
                    xin = bufs[gi][:, 0:xb].bitcast(bf)
                    lc = lcp.tile([P, xb // 2], bf, tag=f"lc{gi}", name=f"lc{gi}")
                    nc.scalar.activation(lc[:], xin, Ln, scale=ESC)
                    # af = x * lc, multiplied in place into lc
                    nc.vector.tensor_mul(lc[:], xin, lc[:])
                    lcg.append(lc)

                for gi, g in enumerate(groups):
                    for t in g:
                        NK = sizes[t]
                        W1 = NK + 2
                        o0 = xoff[t] // 2              # bf16 col offset of x0_t
                        a = wk.tile([P, W1], bf, tag="a")
                        nc.vector.tensor_add(
                            a[:],
                            lcg[gi][:, o0 : o0 + W1],
                            lcg[gi][:, o0 + W1 : o0 + 2 * W1],
                        )
                        gb = bufs[gi]
                        xo, qo = xoff[t], qoff[t]
                        first = t == 0
                        last = t == len(sizes) - 1
                        last_c = NK - MMCH
                        for c in range(0, NK, MMCH):
                            fc = first and c == 0
                            lastc = last and c == last_c
                            x0c = gb[:, xo + 2 * c : xo + 2 * c + 2 * MMCH]
                            x1c = gb[
                                :, xo + 2 * W1 + 2 * c : xo + 2 * W1 + 2 * c + 2 * MMCH
                            ]
                            nc.tensor.matmul(
                                ps1[:], x0c.bitcast(bf),
                                a[:, c + 1 : c + 1 + MMCH],
                                start=fc, stop=False,
                            )
                            nc.tensor.matmul(
                                ps1[:], x1c.bitcast(bf),
                                a[:, c + 1 : c + 1 + MMCH],
                                start=False, stop=lastc,
                            )
                            nc.tensor.matmul(
                                ps2[:],
                                gb[:, qo + c : qo + c + MMCH],
                                gb[:, qo + c + 1 : qo + c + 1 + MMCH],
                                start=fc, stop=lastc,
                            )

            # epilogue (once per launch): PSUM copy-out + alpha terms; the
            # host takes the diagonal traces in f64
            outsb = cs.tile([P, 2 * MMCH + 4], f32, tag="outsb")
            nc.gpsimd.memset(outsb[:], 0.0)
            nc.vector.tensor_copy(outsb[:, 0:MMCH], ps1[:])
            nc.vector.tensor_copy(outsb[:, MMCH : 2 * MMCH], ps2[:])

            t0 = cs.tile([P, 2], bf, tag="t0")
            nc.sync.dma_start(t0[:], f0d[:])
            l0 = cs.tile([P, 2], bf, tag="l0")
            nc.scalar.activation(l0[:, 0:1], t0[:, 0:1], Ln, scale=1.0 / (1.0 - ALPHA))
            nc.scalar.activation(l0[:, 1:2], t0[:, 1:2], Ln, scale=1.0 / ALPHA)
            e3 = cs.tile([P, 2], f32, tag="e3")
            nc.vector.tensor_mul(e3[:], t0[:], l0[:])
            nc.vector.tensor_reduce(
                outsb[:, 2 * MMCH : 2 * MMCH + 1], e3[:], mybir.AxisListType.X, add
            )
            nc.sync.dma_start(outd[:], outsb[:])
    nc.compile()
    return nc


def _assign_rows(lengths):
    """Greedy LPT balance of rows across cores by packed size (len+1)."""
    order = np.argsort(-lengths)
    loads = np.zeros(NCORES, np.int64)
    rows = [[] for _ in range(NCORES)]
    for r in order:
        c = int(np.argmin(loads))
        rows[c].append(int(r))
        loads[c] += int(lengths[r]) + 1
    return rows, loads


def _pack_plane(plane_rows, width):
    """Pack a list of row f32 arrays into [P, width+2] with halo/pad."""
    flat = np.full(P * width, EPS, np.float32)
    pos = 0
    for seg in plane_rows:
        L = seg.shape[0]
        flat[pos : pos + L] = seg
        pos += L + 1                          # eps separator
    arr = np.empty((P, width + 2), np.float32)
    arr[:, 1 : width + 1] = flat.reshape(P, width)
    arr[0, 0] = EPS                           # virtual past for first row
    arr[1:, 0] = arr[:-1, width]              # halo: previous flat element
    arr[:, width + 1] = EPS                   # lookahead pad (never a current)
    return arr


def _prep_core(p0, p1, lengths, rows, width):
    """Pack valid prefixes of `rows` into the two byte-typed group buffers."""
    arr0 = _pack_plane([p0[r, : lengths[r]] for r in rows], width)
    arr1 = _pack_plane([p1[r, : lengths[r]] for r in rows], width)
    arrq = arr0 - arr1
    sizes, groups, xoff, qoff, gbytes = _layout(width)
    starts = [sum(sizes[:i]) for i in range(len(sizes))]
    maps = {}
    for gi, g in enumerate(groups):
        if not g:
            continue
        gbuf = np.zeros((P, gbytes[gi]), np.uint8)
        for t in g:
            NK = sizes[t]
            W1 = NK + 2
            st = starts[t]
            xb = gbuf[:, xoff[t] : xoff[t] + 4 * W1].view(BF)
            xb[:, 0:W1] = arr0[:, st : st + W1].astype(BF)
            xb[:, W1 : 2 * W1] = arr1[:, st : st + W1].astype(BF)
            qb = gbuf[:, qoff[t] : qoff[t] + W1].view(F8)
            qb[:] = arrq[:, st : st + W1].astype(F8)
        maps["gA" if gi == 0 else "gB"] = gbuf.view(F8)
    f0 = np.empty((P, 2), np.float32)
    f0[:, 0] = 1.0 - ALPHA                    # pad rows contribute exactly 0
    f0[:, 1] = ALPHA
    nr = len(rows)
    f0[:nr, 0] = p0[rows, 0]
    f0[:nr, 1] = p1[rows, 0]
    maps["f0"] = f0.astype(BF)
    return maps


def kernel(posterior, length):
    post = np.asarray(posterior, dtype=np.float32)
    ln = np.asarray(length).astype(np.int64)
    assert post.shape == (B, S, 2), post.shape
    lengths = np.clip(ln, 1, S)

    p0 = np.ascontiguousarray(post[..., 0])
    p1 = np.ascontiguousarray(post[..., 1])
    rows, loads = _assign_rows(lengths)
    # common packed width per partition, rounded up to MMCH granularity
    wmax = int(np.ceil(loads.max() / P))
    width = max(MMCH, -(-wmax // MMCH) * MMCH)

    in_maps = [
        _prep_core(p0, p1, lengths, rows[c], width) for c in range(NCORES)
    ]

    if width not in _BUILT:
        _BUILT[width] = _build(width)
    res = run_bass_kernel_spmd(_BUILT[width], in_maps, core_ids=list(range(NCORES)))

    total = np.float64(0.0)
    for c, r in enumerate(res.results):
        acc = np.asarray(r["acc"], np.float64)
        t1 = np.trace(acc[:, 0:MMCH])
        t2 = np.trace(acc[:, MMCH : 2 * MMCH])
        total += t1 - C2 * t2 + acc[: len(rows[c]), 2 * MMCH].sum()
    return np.float32(total / B)
